# revision 47
# baseline (speedup 1.0000x reference)
"""Multi-head attention Trainium2 kernel, 8-core SPMD.

Sharding: 16 (batch, head) pairs over 8 cores -> each core computes 2 heads
of one batch and returns a partial [N, D] output (bf16); host sums 4
partials per batch in fp32.

v9 dataflow (v3 + startup/PSUM restructure + global PV trickle):
  host:  xq/xv slab-major [128, NS, DC, 512]; xk mc-major [128, MC, DC, 128]
         wall = [wk|wq|wv|wp] (scale folded into Wq on host), all bf16
  startup: fine-grained DMA (wk 128KB, k mc0/1, wq, q0 in 2 dc-halves) so
         the first S matmul is gated on ~1.2MB of critical bytes with the
         projection chain overlapped against later transfers.
  unit (qq, mc): both heads' S via concurrent PE row-tiles (K=64):
    S_h  = KT_h.T @ QT_h -> s2[:, h*512:]    [128 m, 1024] PSUM fp32
    P    = exp(s2)        one ACT op -> bf16
    O_h += [V_h | 1].T @ P_h                 [65, 512] PSUM, accum over mc
  The ACT queue (64 exps x ~1.0us + sem waits) is the roofline; the TRN2
  PE p-state throttle (1.2GHz until ~3us continuously busy, then 2.4GHz)
  makes every PE stall double-cost, so the whole schedule exists to keep
  per-unit PE work level and the in-order PE queue free of head-of-line
  waits on DVE results:
  - S pairs are emitted one unit AHEAD (right behind the previous exp) so
    they sit in front of pops/PVs in the PE queue.
  - PSUM tags: s2 (2 bufs x 2 banks) | pp proj/rb/f (2 x 1 bank) | o
    (2 x 1 bank) = 8 banks; proj tiles don't share the s2 ring.
  - global PV trickle: qq0 banks its PVs (lag 11; it already carries the
    K/V/Q1 projection work) and the backlog drains ~1.2/unit through the
    under-loaded later quarters (pt pool 15 bufs holds the P backlog and
    keeps exp's pt-slot wait far in the past).
  - at each quarter's last PV, the normalize chain runs promptly (cpy +
    rowsum-broadcast inline, recip/mul next unit, freeing psO's slot) and
    the 4 output-projection finals are spaced 2 units apart so an f
    matmul's pp-slot wait (two-ago f's ob cast) never blocks the queue.
  u-chain: r = O[64]; rb = ones.T @ r (bcast); un = O[0:64] * 1/rb
  out[q,:] = un.T @ Wp (head sum via 128-contraction), bf16 store
"""

import os
import sys

import numpy as np

sys.path.insert(0, "/opt/trn_rl_repo")

import ml_dtypes
from contextlib import ExitStack

import concourse.bass as bass
import concourse.mybir as mybir
import concourse.tile as tile
from concourse import bacc
from concourse.bass_utils import run_bass_kernel_spmd

B, N, D, H, HS = 2, 2048, 512, 8, 64
NCORES = 8
BF16 = mybir.dt.bfloat16
FP32 = mybir.dt.float32
nbf16 = ml_dtypes.bfloat16

DC = D // 128  # 4 d-chunks
MC = N // 128  # 16 m-chunks
QQ = 4  # q quarters
QV = N // QQ  # 512 q per quarter
SLAB = 512
NS = N // SLAB


def build_nc(finalize=True):
    nc = bacc.Bacc()
    xq = nc.dram_tensor("xq", [128, NS, DC, SLAB], BF16, kind="ExternalInput")
    xk = nc.dram_tensor("xk", [128, MC, DC, 128], BF16, kind="ExternalInput")
    xv = nc.dram_tensor("xv", [128, NS, DC, SLAB], BF16, kind="ExternalInput")
    wall = nc.dram_tensor("wall", [128, 4 * 512], BF16, kind="ExternalInput")
    out = nc.dram_tensor("out", [N, D], BF16, kind="ExternalOutput")

    with tile.TileContext(nc) as tc, ExitStack() as ctx:
        consts = ctx.enter_context(tc.tile_pool(name="consts", bufs=1))
        xt_pool = ctx.enter_context(tc.tile_pool(name="xt", bufs=1))
        kq_pool = ctx.enter_context(tc.tile_pool(name="kq", bufs=1))
        pt_pool = ctx.enter_context(tc.tile_pool(name="pt", bufs=15))
        un_pool = ctx.enter_context(tc.tile_pool(name="un", bufs=2))
        rs_pool = ctx.enter_context(tc.tile_pool(name="rs", bufs=2))
        rb_pool = ctx.enter_context(tc.tile_pool(name="rb", bufs=2))
        ob_pool = ctx.enter_context(tc.tile_pool(name="ob", bufs=3))
        psA = ctx.enter_context(tc.tile_pool(name="psA", bufs=2, space="PSUM"))
        psO = ctx.enter_context(tc.tile_pool(name="psO", bufs=2, space="PSUM"))

        # input tiles + fine-grained DMA in dependency-priority order (all
        # on the sync HWDGE queue: the 16 SDMA engines round-robin, so one
        # strictly-ordered stream keeps the critical prefix first)
        wall_s = consts.tile([128, 4 * 512], BF16, tag="wall_s")
        xk_s = xt_pool.tile([128, MC, DC, 128], BF16, tag="xt_k", name="xk_s")
        xq_s = xt_pool.tile([128, NS, DC, SLAB], BF16, tag="xt_q", name="xq_s")
        xv_s = xt_pool.tile([128, NS, DC, SLAB], BF16, tag="xt_v", name="xv_s")

        nc.sync.dma_start(out=wall_s[:, 0:512], in_=wall[:, 0:512])  # wk
        nc.sync.dma_start(out=xk_s[:, 0:2], in_=xk[:, 0:2])  # k mc0-1
        nc.sync.dma_start(out=wall_s[:, 512:1024], in_=wall[:, 512:1024])  # wq
        nc.sync.dma_start(out=xq_s[:, 0, 0:2], in_=xq[:, 0, 0:2])  # q0 dc01
        nc.sync.dma_start(out=xq_s[:, 0, 2:4], in_=xq[:, 0, 2:4])  # q0 dc23
        nc.sync.dma_start(out=xk_s[:, 2:4], in_=xk[:, 2:4])  # k mc2-3
        nc.sync.dma_start(out=xk_s[:, 4:8], in_=xk[:, 4:8])
        nc.sync.dma_start(out=wall_s[:, 1024:2048], in_=wall[:, 1024:2048])
        nc.sync.dma_start(out=xv_s[:, 0:1], in_=xv[:, 0:1])
        nc.sync.dma_start(out=xk_s[:, 8:16], in_=xk[:, 8:16])
        nc.sync.dma_start(out=xv_s[:, 1:2], in_=xv[:, 1:2])
        nc.sync.dma_start(out=xq_s[:, 1:2], in_=xq[:, 1:2])
        nc.sync.dma_start(out=xv_s[:, 2:3], in_=xv[:, 2:3])
        nc.sync.dma_start(out=xv_s[:, 3:4], in_=xv[:, 3:4])
        nc.sync.dma_start(out=xq_s[:, 2:4], in_=xq[:, 2:4])

        def w_slice(name, dc):
            off = {"k": 0, "q": 512, "v": 1024}[name] + dc * 128
            return wall_s[:, off : off + 128]

        wp_s = wall_s[:, 1536:2048]

        # Vn: [128 m, mc, head, 65]; col HS = ones (rowsum trick)
        vn = consts.tile([128, MC, 2, HS + 1], BF16, tag="vn")
        nc.gpsimd.memset(vn[:, :, :, HS : HS + 1], 1.0)
        ones_row = consts.tile([HS + 1, HS], BF16, tag="ones_row")
        nc.gpsimd.memset(ones_row[HS : HS + 1, :], 1.0)
        # warm the ACT exp table while DMAs stream
        warm = consts.tile([1, 1], BF16, tag="warm")
        nc.scalar.activation(
            warm[:], ones_row[HS : HS + 1, 0:1],
            mybir.ActivationFunctionType.Exp,
        )

        kt2 = kq_pool.tile([128, N], BF16, tag="kt2", name="kt2")
        qt2 = kq_pool.tile([128, N], BF16, tag="qt2", name="qt2")

        def kproj_mc(mc):
            pr = psA.tile([128, 128], FP32, tag="pp", name="pr")
            for dc in range(DC):
                nc.tensor.matmul(
                    pr[:], w_slice("k", dc), xk_s[:, mc, dc, :],
                    start=(dc == 0), stop=(dc == DC - 1),
                )
            nc.vector.tensor_copy(kt2[:, mc * 128 : (mc + 1) * 128], pr[:])

        def kslab_parts(j):
            st = {}

            def part_a():
                st["pr"] = psA.tile([128, SLAB], FP32, tag="pp", name="pr")
                for dc in (0, 1):
                    nc.tensor.matmul(
                        st["pr"][:], w_slice("k", dc),
                        xk_s[:, 4 * j : 4 * j + 4, dc, :],
                        start=(dc == 0), stop=False,
                    )

            def part_b():
                for dc in (2, 3):
                    nc.tensor.matmul(
                        st["pr"][:], w_slice("k", dc),
                        xk_s[:, 4 * j : 4 * j + 4, dc, :],
                        start=False, stop=(dc == 3),
                    )
                nc.vector.tensor_copy(
                    kt2[:, j * SLAB : (j + 1) * SLAB], st["pr"][:]
                )

            return [part_a, part_b]

        def qslab_parts(j):
            st = {}

            def part_a():
                st["pr"] = psA.tile([128, SLAB], FP32, tag="pp", name="pr")
                for dc in (0, 1):
                    nc.tensor.matmul(
                        st["pr"][:], w_slice("q", dc), xq_s[:, j, dc, :],
                        start=(dc == 0), stop=False,
                    )

            def part_b():
                for dc in (2, 3):
                    nc.tensor.matmul(
                        st["pr"][:], w_slice("q", dc), xq_s[:, j, dc, :],
                        start=False, stop=(dc == 3),
                    )
                nc.vector.tensor_copy(
                    qt2[:, j * SLAB : (j + 1) * SLAB], st["pr"][:]
                )

            return [part_a, part_b]

        def vproj_half(jh):
            # V in natural [m, hs2] orientation: 2 m-chunks per item
            vp = psA.tile([128, 256], FP32, tag="pp", name="vp")
            for m2 in range(2):
                mc = jh * 2 + m2
                for dc in range(DC):
                    nc.tensor.matmul(
                        vp[:, m2 * 128 : (m2 + 1) * 128],
                        xv_s[:, mc // 4, dc, (mc % 4) * 128 : (mc % 4 + 1) * 128],
                        w_slice("v", dc),
                        start=(dc == 0), stop=(dc == DC - 1),
                    )
            nc.vector.tensor_copy(
                vn[:, jh * 2 : (jh + 1) * 2, :, 0:HS],
                vp[:].rearrange("p (m h c) -> p m h c", m=2, h=2),
            )

        def emit_final(qq_, un_t, c, last=False):
            f_ps = psA.tile([128, D], FP32, tag="pp", name="f_ps")
            nc.tensor.matmul(
                f_ps[:], un_t[:, c * 128 : (c + 1) * 128], wp_s[:],
                start=True, stop=True,
            )
            ob = ob_pool.tile([128, D], BF16, tag="ob", name="ob")
            if last and c % 2 == 0:
                # after the final exp both ACT and DVE go idle; alternate
                # the tail casts between them so neither queue serializes
                nc.scalar.copy(ob[:], f_ps[:])
            else:
                nc.vector.tensor_copy(ob[:], f_ps[:])
            nc.sync.dma_start(
                out=out[qq_ * QV + c * 128 : qq_ * QV + (c + 1) * 128, :],
                in_=ob[:],
            )

        def tail_steps(qq_, o_ps_, un_t):
            # r = rowsums (row 64 of o); broadcast to 64 partitions via one
            # ones.T @ r matmul per head; un = o[0:64] * 1/rb.
            r_sb = rs_pool.tile([HS + 1, 2, QV], BF16, tag="r", name="r_sb")
            rb_sb = rb_pool.tile([HS, 2, QV], FP32, tag="rb", name="rb_sb")
            rb_ps = [None, None]

            def cpy():
                for h in range(2):
                    nc.vector.tensor_copy(
                        r_sb[HS : HS + 1, h, :], o_ps_[h][HS : HS + 1, :]
                    )

            def rbmm():
                for h in range(2):
                    rb_ps[h] = psA.tile([HS, QV], FP32, tag="pp", name="rb_ps")
                    nc.tensor.matmul(
                        rb_ps[h][:], ones_row[HS : HS + 1, :],
                        r_sb[HS : HS + 1, h, :], start=True, stop=True,
                    )

            def recip():
                for h in range(2):
                    nc.vector.reciprocal_approx_fast(
                        rb_sb[:, h, :], rb_ps[h][:]
                    )

            def muls():
                for h in range(2):
                    nc.vector.tensor_mul(
                        un_t[HS * h : HS * h + HS, :],
                        o_ps_[h][0:HS, :], rb_sb[:, h, :],
                    )

            return [cpy, rbmm, recip, muls]

        def lag_of(u):
            # global PV backlog target: qq0 banks its PVs (it already
            # carries all K/V/Q1 projection work), the backlog then drains
            # gently (~1.2 PV/unit) through the under-loaded later quarters
            if u < 16:
                return 11
            if u < 56:
                return max(2, 11 - (u - 16) * 9 // 40)
            if u < 62:
                return 2
            return 1

        def tail_last(qq_, o_ps_, un_t):
            r_sb = rs_pool.tile([HS + 1, 2, QV], BF16, tag="r", name="r_sb")
            rb_sb = rb_pool.tile([HS, 2, QV], FP32, tag="rb", name="rb_sb")
            HF = QV // 2
            # split the rowsum copies across ACT and DVE (both idle here)
            nc.scalar.copy(r_sb[HS : HS + 1, 0, :], o_ps_[0][HS : HS + 1, :])
            nc.vector.tensor_copy(
                r_sb[HS : HS + 1, 1, :], o_ps_[1][HS : HS + 1, :]
            )
            rb_ps = []
            for h in range(2):
                rp = psA.tile([HS, QV], FP32, tag="pp", name="rb_ps")
                nc.tensor.matmul(
                    rp[:], ones_row[HS : HS + 1, :], r_sb[HS : HS + 1, h, :],
                    start=True, stop=True,
                )
                rb_ps.append(rp)

            def recip(lo, hi):
                for h in range(2):
                    nc.vector.reciprocal_approx_fast(
                        rb_sb[:, h, lo:hi], rb_ps[h][:, lo:hi]
                    )

            def mul(h, lo, hi):
                nc.vector.tensor_mul(
                    un_t[HS * h : HS * h + HS, lo:hi],
                    o_ps_[h][0:HS, lo:hi], rb_sb[:, h, lo:hi],
                )

            # half-granularity so the first final MMs start while the
            # second half of the normalization still runs
            recip(0, HF)
            mul(0, 0, HF)
            mul(1, 0, HF)
            recip(HF, QV)
            emit_final(qq_, un_t, 0, last=True)
            mul(0, HF, QV)
            mul(1, HF, QV)
            emit_final(qq_, un_t, 1, last=True)
            emit_final(qq_, un_t, 2, last=True)
            emit_final(qq_, un_t, 3, last=True)

        # ---- startup: minimal gate for the first S matmul (wk + k mc0/1
        # + wq + q slab0); kmc2/3 run right after S_0 so units 0-2 never
        # wait on a cold projection chain
        kproj_mc(0)
        q0 = qslab_parts(0)
        q0[0]()
        kproj_mc(1)
        q0[1]()

        k1 = kslab_parts(1)
        k2 = kslab_parts(2)
        k3 = kslab_parts(3)
        q1 = qslab_parts(1)
        q2 = qslab_parts(2)
        q3 = qslab_parts(3)
        deferred = [
            lambda: kproj_mc(2),                 # unit 0
            lambda: (kproj_mc(3), k1[0]()),      # 1
            k1[1],                               # 2   (S_4 emitted unit 3)
            k2[0],                               # 3
            k2[1],                               # 4   (S_8 emitted unit 7)
            lambda: vproj_half(0),               # 5
            lambda: vproj_half(1),               # 6
            k3[0],                               # 7
            k3[1],                               # 8   (S_12 emitted unit 11)
            lambda: vproj_half(2),               # 9
            lambda: vproj_half(3),               # 10
            lambda: vproj_half(4),               # 11
            lambda: (q1[0](), q1[1]()),          # 12  (S_0' emitted unit 15)
            lambda: vproj_half(5),               # 13
            lambda: vproj_half(6),               # 14
            lambda: vproj_half(7),               # 15
            q2[0],                               # 16
            q2[1],                               # 17
            q3[0],                               # 18
            q3[1],                               # 19
        ]
        btail = []

        # S pairs are emitted one unit AHEAD (right behind the previous
        # exp in program order) so they sit in front of pops/PVs in the
        # in-order PE queue and the ACT engine is never data-starved by
        # queued-behind filler work.
        s2d = {}

        def emit_S(qq_, mc):
            s2 = psA.tile([128, 1024], FP32, tag="s2", name="s2")
            for h in range(2):
                nc.tensor.matmul(
                    s2[:, h * QV : (h + 1) * QV],
                    kt2[h * HS : (h + 1) * HS, mc * 128 : (mc + 1) * 128],
                    qt2[h * HS : (h + 1) * HS, qq_ * QV : (qq_ + 1) * QV],
                    start=True,
                    stop=True,
                    tile_position=(h * HS, 0),
                )
            s2d[(qq_, mc)] = s2

        emit_S(0, 0)
        # both s2 slots are free at startup, so S_1 can be pre-emitted
        # with no waits — exp_1 then never stalls behind the kmc2/kmc3
        # projection chains popped at unit 0
        emit_S(0, 1)

        # global PV trickle: pend spans quarter boundaries; each entry is
        # (is_last_of_quarter, pv_thunk, finish_thunk). When a quarter's
        # last PV pops, its normalize chain runs inline (freeing the psO
        # slot) and the 4 output-projection finals go to btail (1/unit);
        # PV pops then pause 2 units so the next quarter's first PV never
        # head-blocks on the just-emitted DVE chain.
        pend = []
        cooldown = [0]

        def pop_pend(limit=2):
            popped = 0
            while pend and popped < limit:
                fin, pvt, fint = pend[0]
                pvt()
                pend.pop(0)
                popped += 1
                if fin:
                    fint()
                    cooldown[0] = 3
                    break

        for qq in range(QQ):
            o_ps = [
                psO.tile([HS + 1, QV], FP32, tag="o", name=f"o{h}")
                for h in range(2)
            ]
            un2 = un_pool.tile([128, QV], BF16, tag="un", name="un2")

            def pv(mc, p_sb, o_ps_=o_ps):
                for h in range(2):
                    nc.tensor.matmul(
                        o_ps_[h][:],
                        vn[:, mc, h, :],
                        p_sb[:, h * QV : (h + 1) * QV],
                        start=(mc == 0),
                        stop=(mc == MC - 1),
                    )

            def fin_q(qq_=qq, o_ps_=o_ps, un_t=un2):
                if qq_ < QQ - 1:
                    cpy, rbmm, recip, muls = tail_steps(qq_, o_ps_, un_t)
                    cpy()
                    rbmm()
                    # recip/muls next unit (frees o_ps before the cooldown
                    # ends); finals spaced so each f matmul's pp-slot wait
                    # (two-ago f's ob cast) is long done when it pops
                    fs = [
                        (lambda c=c: emit_final(qq_, un_t, c))
                        for c in range(4)
                    ]
                    btail.extend(
                        [[recip, muls], [], [fs[0]], [], [fs[1]], [],
                         [fs[2]], [], [fs[3]]]
                    )
                else:
                    tail_last(qq_, o_ps_, un_t)

            for mc in range(MC):
                u = qq * MC + mc
                s2 = s2d.pop((qq, mc))
                p_sb = pt_pool.tile([128, 1024], BF16, tag="p", name="p_sb")
                nc.scalar.activation(
                    p_sb[:], s2[:], mybir.ActivationFunctionType.Exp
                )
                if mc < MC - 1:
                    if (qq, mc + 1) not in s2d:
                        emit_S(qq, mc + 1)
                elif qq < QQ - 1:
                    emit_S(qq + 1, 0)
                if btail:
                    for fn in btail.pop(0):
                        fn()
                elif deferred:
                    deferred.pop(0)()
                pend.append(
                    (mc == MC - 1, lambda m=mc, p=p_sb, f=pv: f(m, p), fin_q)
                )
                if cooldown[0] > 0:
                    cooldown[0] -= 1
                elif len(pend) > lag_of(u):
                    cap = 1 if btail else 2
                    pop_pend(min(cap, len(pend) - lag_of(u)))

        # drain: remaining PVs of the last quarter, then its inline tail
        while pend:
            pop_pend(4)
        while btail:
            for fn in btail.pop(0):
                fn()
    if finalize:
        nc.finalize()
    return nc


_NC_CACHE = None


def _get_nc():
    global _NC_CACHE
    if _NC_CACHE is None:
        _NC_CACHE = build_nc()
    return _NC_CACHE


def _prep_xt(x, dt):
    # [N, D] fp32 -> [128, NS, DC, SLAB] slab-major:
    # xt[p, j, c, n'] = x[j*SLAB + n', c*128 + p]
    return np.ascontiguousarray(
        x.reshape(NS, SLAB, DC, 128).transpose(3, 0, 2, 1)
    ).astype(dt)


def _prep_xk(x, dt):
    # [N, D] fp32 -> [128, MC, DC, 128] mc-major:
    # xt[p, mc, c, n'] = x[mc*128 + n', c*128 + p]
    return np.ascontiguousarray(
        x.reshape(MC, 128, DC, 128).transpose(3, 0, 2, 1)
    ).astype(dt)


def _prep_w(w2, dt):
    # [D, 128] -> [128, DC, 128] with w[p, c, h] = w2[c*128+p, h]
    return np.ascontiguousarray(
        w2.reshape(DC, 128, 128).transpose(1, 0, 2)
    ).astype(dt)


def make_in_maps(inputs):
    query = np.asarray(inputs["query"], np.float32)
    key = np.asarray(inputs["key"], np.float32)
    value = np.asarray(inputs["value"], np.float32)
    Wq = np.asarray(inputs["Wq"], np.float32) / np.sqrt(np.float32(HS))
    Wk = np.asarray(inputs["Wk"], np.float32)
    Wv = np.asarray(inputs["Wv"], np.float32)
    Wp = np.asarray(inputs["Wp"], np.float32)

    in_maps = []
    for c in range(NCORES):
        b = c // 4
        h0 = 2 * (c % 4)
        w_all = np.concatenate(
            [
                _prep_w(
                    np.concatenate([W[h0], W[h0 + 1]], axis=1), np.float32
                ).reshape(128, DC * 128)
                for W in (Wk, Wq, Wv)
            ]
            + [np.concatenate([Wp[h0], Wp[h0 + 1]], axis=0)],
            axis=1,
        ).astype(nbf16)
        in_maps.append(
            {
                "xq": _prep_xt(query[b], nbf16),
                "xk": _prep_xk(key[b], nbf16),
                "xv": _prep_xt(value[b], nbf16),
                "wall": np.ascontiguousarray(w_all),
            }
        )
    return in_maps


def kernel(query, key, value, Wq, Wk, Wv, Wp):
    in_maps = make_in_maps(
        dict(query=query, key=key, value=value, Wq=Wq, Wk=Wk, Wv=Wv, Wp=Wp)
    )
    nc = _get_nc()
    res = run_bass_kernel_spmd(nc, in_maps, list(range(NCORES)))
    out = np.zeros((B, N, D), np.float32)
    for c in range(NCORES):
        out[c // 4] += np.asarray(res.results[c]["out"], np.float32)
    return out


if __name__ == "__main__":
    d = np.load("/root/problem/work/ref.npz")
    got = kernel(
        d["query"], d["key"], d["value"], d["Wq"], d["Wk"], d["Wv"], d["Wp"]
    )
    exp = d["expected"]
    rel = np.linalg.norm(got - exp) / np.linalg.norm(exp)
    print("Relative error:", rel)


# revision 49
# speedup vs baseline: 1.0098x; 1.0098x over previous
"""Multi-head attention Trainium2 kernel, 8-core SPMD.

Sharding: 16 (batch, head) pairs over 8 cores -> each core computes 2 heads
of one batch and returns a partial [N, D] output (bf16); host sums 4
partials per batch in fp32.

v9 dataflow (v3 + startup/PSUM restructure + global PV trickle):
  host:  xq/xv slab-major [128, NS, DC, 512]; xk mc-major [128, MC, DC, 128]
         wall = [wk|wq|wv|wp] (scale folded into Wq on host), all bf16
  startup: fine-grained DMA (wk 128KB, k mc0/1, wq, q0 in 2 dc-halves) so
         the first S matmul is gated on ~1.2MB of critical bytes with the
         projection chain overlapped against later transfers.
  unit (qq, mc): both heads' S via concurrent PE row-tiles (K=64):
    S_h  = KT_h.T @ QT_h -> s2[:, h*512:]    [128 m, 1024] PSUM fp32
    P    = exp(s2)        one ACT op -> bf16
    O_h += [V_h | 1].T @ P_h                 [65, 512] PSUM, accum over mc
  The ACT queue (64 exps x ~1.0us + sem waits) is the roofline; the TRN2
  PE p-state throttle (1.2GHz until ~3us continuously busy, then 2.4GHz)
  makes every PE stall double-cost, so the whole schedule exists to keep
  per-unit PE work level and the in-order PE queue free of head-of-line
  waits on DVE results:
  - S pairs are emitted one unit AHEAD (right behind the previous exp) so
    they sit in front of pops/PVs in the PE queue.
  - PSUM tags: s2 (2 bufs x 2 banks) | pp proj/rb/f (2 x 1 bank) | o
    (2 x 1 bank) = 8 banks; proj tiles don't share the s2 ring.
  - global PV trickle: qq0 banks its PVs (lag 11; it already carries the
    K/V/Q1 projection work) and the backlog drains ~1.2/unit through the
    under-loaded later quarters (pt pool 15 bufs holds the P backlog and
    keeps exp's pt-slot wait far in the past).
  - at each quarter's last PV, the normalize chain runs promptly (cpy +
    rowsum-broadcast inline, recip/mul next unit, freeing psO's slot) and
    the 4 output-projection finals are spaced 2 units apart so an f
    matmul's pp-slot wait (two-ago f's ob cast) never blocks the queue.
  u-chain: r = O[64]; rb = ones.T @ r (bcast); un = O[0:64] * 1/rb
  out[q,:] = un.T @ Wp (head sum via 128-contraction), bf16 store
"""

import os
import sys

import numpy as np

sys.path.insert(0, "/opt/trn_rl_repo")

import ml_dtypes
from contextlib import ExitStack

import concourse.bass as bass
import concourse.mybir as mybir
import concourse.tile as tile
from concourse import bacc
from concourse.bass_utils import run_bass_kernel_spmd

B, N, D, H, HS = 2, 2048, 512, 8, 64
NCORES = 8
BF16 = mybir.dt.bfloat16
FP32 = mybir.dt.float32
nbf16 = ml_dtypes.bfloat16

DC = D // 128  # 4 d-chunks
MC = N // 128  # 16 m-chunks
QQ = 4  # q quarters
QV = N // QQ  # 512 q per quarter
SLAB = 512
NS = N // SLAB


def build_nc(finalize=True):
    nc = bacc.Bacc()
    xq = nc.dram_tensor("xq", [128, NS, DC, SLAB], BF16, kind="ExternalInput")
    xk = nc.dram_tensor("xk", [128, MC, DC, 128], BF16, kind="ExternalInput")
    xv = nc.dram_tensor("xv", [128, NS, DC, SLAB], BF16, kind="ExternalInput")
    wall = nc.dram_tensor("wall", [128, 4 * 512], BF16, kind="ExternalInput")
    out = nc.dram_tensor("out", [N, D], BF16, kind="ExternalOutput")

    with tile.TileContext(nc) as tc, ExitStack() as ctx:
        consts = ctx.enter_context(tc.tile_pool(name="consts", bufs=1))
        xt_pool = ctx.enter_context(tc.tile_pool(name="xt", bufs=1))
        kq_pool = ctx.enter_context(tc.tile_pool(name="kq", bufs=1))
        pt_pool = ctx.enter_context(tc.tile_pool(name="pt", bufs=15))
        un_pool = ctx.enter_context(tc.tile_pool(name="un", bufs=2))
        rs_pool = ctx.enter_context(tc.tile_pool(name="rs", bufs=2))
        rb_pool = ctx.enter_context(tc.tile_pool(name="rb", bufs=2))
        ob_pool = ctx.enter_context(tc.tile_pool(name="ob", bufs=3))
        psA = ctx.enter_context(tc.tile_pool(name="psA", bufs=2, space="PSUM"))
        psO = ctx.enter_context(tc.tile_pool(name="psO", bufs=2, space="PSUM"))

        # input tiles + fine-grained DMA in dependency-priority order (all
        # on the sync HWDGE queue: the 16 SDMA engines round-robin, so one
        # strictly-ordered stream keeps the critical prefix first)
        wall_s = consts.tile([128, 4 * 512], BF16, tag="wall_s")
        xk_s = xt_pool.tile([128, MC, DC, 128], BF16, tag="xt_k", name="xk_s")
        xq_s = xt_pool.tile([128, NS, DC, SLAB], BF16, tag="xt_q", name="xq_s")
        xv_s = xt_pool.tile([128, NS, DC, SLAB], BF16, tag="xt_v", name="xv_s")

        nc.sync.dma_start(out=wall_s[:, 0:512], in_=wall[:, 0:512])  # wk
        nc.sync.dma_start(out=xk_s[:, 0:2], in_=xk[:, 0:2])  # k mc0-1
        nc.sync.dma_start(out=wall_s[:, 512:1024], in_=wall[:, 512:1024])  # wq
        nc.sync.dma_start(out=xq_s[:, 0, 0:2], in_=xq[:, 0, 0:2])  # q0 dc01
        nc.sync.dma_start(out=xq_s[:, 0, 2:4], in_=xq[:, 0, 2:4])  # q0 dc23
        nc.sync.dma_start(out=xk_s[:, 2:4], in_=xk[:, 2:4])  # k mc2-3
        nc.sync.dma_start(out=xk_s[:, 4:8], in_=xk[:, 4:8])
        nc.sync.dma_start(out=wall_s[:, 1024:2048], in_=wall[:, 1024:2048])
        nc.sync.dma_start(out=xv_s[:, 0:1], in_=xv[:, 0:1])
        nc.sync.dma_start(out=xk_s[:, 8:16], in_=xk[:, 8:16])
        nc.sync.dma_start(out=xv_s[:, 1:2], in_=xv[:, 1:2])
        nc.sync.dma_start(out=xq_s[:, 1:2], in_=xq[:, 1:2])
        nc.sync.dma_start(out=xv_s[:, 2:3], in_=xv[:, 2:3])
        nc.sync.dma_start(out=xv_s[:, 3:4], in_=xv[:, 3:4])
        nc.sync.dma_start(out=xq_s[:, 2:4], in_=xq[:, 2:4])

        def w_slice(name, dc):
            off = {"k": 0, "q": 512, "v": 1024}[name] + dc * 128
            return wall_s[:, off : off + 128]

        wp_s = wall_s[:, 1536:2048]

        # Vn: [128 m, mc, head, 65]; col HS = ones (rowsum trick)
        vn = consts.tile([128, MC, 2, HS + 1], BF16, tag="vn")
        nc.gpsimd.memset(vn[:, :, :, HS : HS + 1], 1.0)
        ones_row = consts.tile([HS + 1, HS], BF16, tag="ones_row")
        nc.gpsimd.memset(ones_row[HS : HS + 1, :], 1.0)
        # warm the ACT exp table while DMAs stream
        warm = consts.tile([1, 1], BF16, tag="warm")
        nc.scalar.activation(
            warm[:], ones_row[HS : HS + 1, 0:1],
            mybir.ActivationFunctionType.Exp,
        )

        kt2 = kq_pool.tile([128, N], BF16, tag="kt2", name="kt2")
        qt2 = kq_pool.tile([128, N], BF16, tag="qt2", name="qt2")

        def kproj_mc(mc):
            pr = psA.tile([128, 128], FP32, tag="pp", name="pr")
            for dc in range(DC):
                nc.tensor.matmul(
                    pr[:], w_slice("k", dc), xk_s[:, mc, dc, :],
                    start=(dc == 0), stop=(dc == DC - 1),
                )
            nc.vector.tensor_copy(kt2[:, mc * 128 : (mc + 1) * 128], pr[:])

        def kslab_parts(j):
            st = {}

            def part_a():
                st["pr"] = psA.tile([128, SLAB], FP32, tag="pp", name="pr")
                for dc in (0, 1):
                    nc.tensor.matmul(
                        st["pr"][:], w_slice("k", dc),
                        xk_s[:, 4 * j : 4 * j + 4, dc, :],
                        start=(dc == 0), stop=False,
                    )

            def part_b():
                for dc in (2, 3):
                    nc.tensor.matmul(
                        st["pr"][:], w_slice("k", dc),
                        xk_s[:, 4 * j : 4 * j + 4, dc, :],
                        start=False, stop=(dc == 3),
                    )
                nc.vector.tensor_copy(
                    kt2[:, j * SLAB : (j + 1) * SLAB], st["pr"][:]
                )

            return [part_a, part_b]

        def qslab_parts(j):
            st = {}

            def part_a():
                st["pr"] = psA.tile([128, SLAB], FP32, tag="pp", name="pr")
                for dc in (0, 1):
                    nc.tensor.matmul(
                        st["pr"][:], w_slice("q", dc), xq_s[:, j, dc, :],
                        start=(dc == 0), stop=False,
                    )

            def part_b():
                for dc in (2, 3):
                    nc.tensor.matmul(
                        st["pr"][:], w_slice("q", dc), xq_s[:, j, dc, :],
                        start=False, stop=(dc == 3),
                    )
                nc.vector.tensor_copy(
                    qt2[:, j * SLAB : (j + 1) * SLAB], st["pr"][:]
                )

            return [part_a, part_b]

        def vproj_half(jh):
            # V in natural [m, hs2] orientation: 2 m-chunks per item
            vp = psA.tile([128, 256], FP32, tag="pp", name="vp")
            for m2 in range(2):
                mc = jh * 2 + m2
                for dc in range(DC):
                    nc.tensor.matmul(
                        vp[:, m2 * 128 : (m2 + 1) * 128],
                        xv_s[:, mc // 4, dc, (mc % 4) * 128 : (mc % 4 + 1) * 128],
                        w_slice("v", dc),
                        start=(dc == 0), stop=(dc == DC - 1),
                    )
            nc.vector.tensor_copy(
                vn[:, jh * 2 : (jh + 1) * 2, :, 0:HS],
                vp[:].rearrange("p (m h c) -> p m h c", m=2, h=2),
            )

        def emit_final(qq_, un_t, c, last=False):
            f_ps = psA.tile([128, D], FP32, tag="pp", name="f_ps")
            nc.tensor.matmul(
                f_ps[:], un_t[:, c * 128 : (c + 1) * 128], wp_s[:],
                start=True, stop=True,
            )
            ob = ob_pool.tile([128, D], BF16, tag="ob", name="ob")
            if last:
                # the ACT engine is idle after the final exp; use it for
                # the tail casts so the DVE chain isn't the critical path
                nc.scalar.copy(ob[:], f_ps[:])
            else:
                nc.vector.tensor_copy(ob[:], f_ps[:])
            nc.sync.dma_start(
                out=out[qq_ * QV + c * 128 : qq_ * QV + (c + 1) * 128, :],
                in_=ob[:],
            )

        def tail_steps(qq_, o_ps_, un_t):
            # r = rowsums (row 64 of o); broadcast to 64 partitions via one
            # ones.T @ r matmul per head; un = o[0:64] * 1/rb.
            r_sb = rs_pool.tile([HS + 1, 2, QV], BF16, tag="r", name="r_sb")
            rb_sb = rb_pool.tile([HS, 2, QV], FP32, tag="rb", name="rb_sb")
            rb_ps = [None, None]

            def cpy():
                for h in range(2):
                    nc.vector.tensor_copy(
                        r_sb[HS : HS + 1, h, :], o_ps_[h][HS : HS + 1, :]
                    )

            def rbmm():
                for h in range(2):
                    rb_ps[h] = psA.tile([HS, QV], FP32, tag="pp", name="rb_ps")
                    nc.tensor.matmul(
                        rb_ps[h][:], ones_row[HS : HS + 1, :],
                        r_sb[HS : HS + 1, h, :], start=True, stop=True,
                    )

            def recip():
                for h in range(2):
                    nc.vector.reciprocal_approx_fast(
                        rb_sb[:, h, :], rb_ps[h][:]
                    )

            def muls():
                for h in range(2):
                    nc.vector.tensor_mul(
                        un_t[HS * h : HS * h + HS, :],
                        o_ps_[h][0:HS, :], rb_sb[:, h, :],
                    )

            return [cpy, rbmm, recip, muls]

        def lag_of(u):
            # global PV backlog target: qq0 banks its PVs (it already
            # carries all K/V/Q1 projection work), the backlog then drains
            # gently (~1.2 PV/unit) through the under-loaded later quarters
            if u < 16:
                return 11
            if u < 56:
                return max(2, 11 - (u - 16) * 9 // 40)
            if u < 62:
                return 2
            return 1

        def tail_last(qq_, o_ps_, un_t):
            r_sb = rs_pool.tile([HS + 1, 2, QV], BF16, tag="r", name="r_sb")
            rb_sb = rb_pool.tile([HS, 2, QV], FP32, tag="rb", name="rb_sb")
            HF = QV // 2
            for h in range(2):
                nc.scalar.copy(
                    r_sb[HS : HS + 1, h, :], o_ps_[h][HS : HS + 1, :]
                )
            rb_ps = []
            for h in range(2):
                rp = psA.tile([HS, QV], FP32, tag="pp", name="rb_ps")
                nc.tensor.matmul(
                    rp[:], ones_row[HS : HS + 1, :], r_sb[HS : HS + 1, h, :],
                    start=True, stop=True,
                )
                rb_ps.append(rp)

            def recip(lo, hi):
                for h in range(2):
                    nc.vector.reciprocal_approx_fast(
                        rb_sb[:, h, lo:hi], rb_ps[h][:, lo:hi]
                    )

            def mul(h, lo, hi):
                nc.vector.tensor_mul(
                    un_t[HS * h : HS * h + HS, lo:hi],
                    o_ps_[h][0:HS, lo:hi], rb_sb[:, h, lo:hi],
                )

            # half-granularity so the first final MMs start while the
            # second half of the normalization still runs
            recip(0, HF)
            mul(0, 0, HF)
            mul(1, 0, HF)
            recip(HF, QV)
            emit_final(qq_, un_t, 0, last=True)
            mul(0, HF, QV)
            mul(1, HF, QV)
            emit_final(qq_, un_t, 1, last=True)
            emit_final(qq_, un_t, 2, last=True)
            emit_final(qq_, un_t, 3, last=True)

        # ---- startup: minimal gate for the first S matmul (wk + k mc0/1
        # + wq + q slab0); kmc2/3 run right after S_0 so units 0-2 never
        # wait on a cold projection chain
        kproj_mc(0)
        q0 = qslab_parts(0)
        q0[0]()
        kproj_mc(1)
        q0[1]()

        k1 = kslab_parts(1)
        k2 = kslab_parts(2)
        k3 = kslab_parts(3)
        q1 = qslab_parts(1)
        q2 = qslab_parts(2)
        q3 = qslab_parts(3)
        deferred = [
            lambda: kproj_mc(2),                 # unit 0
            lambda: (kproj_mc(3), k1[0]()),      # 1
            k1[1],                               # 2   (S_4 emitted unit 3)
            k2[0],                               # 3
            k2[1],                               # 4   (S_8 emitted unit 7)
            lambda: vproj_half(0),               # 5
            lambda: vproj_half(1),               # 6
            k3[0],                               # 7
            k3[1],                               # 8   (S_12 emitted unit 11)
            lambda: vproj_half(2),               # 9
            lambda: vproj_half(3),               # 10
            lambda: vproj_half(4),               # 11
            lambda: (q1[0](), q1[1]()),          # 12  (S_0' emitted unit 15)
            lambda: vproj_half(5),               # 13
            lambda: vproj_half(6),               # 14
            lambda: vproj_half(7),               # 15
            q2[0],                               # 16
            q2[1],                               # 17
            q3[0],                               # 18
            q3[1],                               # 19
        ]
        btail = []

        # S pairs are emitted one unit AHEAD (right behind the previous
        # exp in program order) so they sit in front of pops/PVs in the
        # in-order PE queue and the ACT engine is never data-starved by
        # queued-behind filler work.
        s2d = {}

        def emit_S(qq_, mc):
            s2 = psA.tile([128, 1024], FP32, tag="s2", name="s2")
            for h in range(2):
                nc.tensor.matmul(
                    s2[:, h * QV : (h + 1) * QV],
                    kt2[h * HS : (h + 1) * HS, mc * 128 : (mc + 1) * 128],
                    qt2[h * HS : (h + 1) * HS, qq_ * QV : (qq_ + 1) * QV],
                    start=True,
                    stop=True,
                    tile_position=(h * HS, 0),
                )
            s2d[(qq_, mc)] = s2

        emit_S(0, 0)
        # both s2 slots are free at startup, so S_1 can be pre-emitted
        # with no waits — exp_1 then never stalls behind the kmc2/kmc3
        # projection chains popped at unit 0
        emit_S(0, 1)

        # global PV trickle: pend spans quarter boundaries; each entry is
        # (is_last_of_quarter, pv_thunk, finish_thunk). When a quarter's
        # last PV pops, its normalize chain runs inline (freeing the psO
        # slot) and the 4 output-projection finals go to btail (1/unit);
        # PV pops then pause 2 units so the next quarter's first PV never
        # head-blocks on the just-emitted DVE chain.
        pend = []
        cooldown = [0]

        def pop_pend(limit=2):
            popped = 0
            while pend and popped < limit:
                fin, pvt, fint = pend[0]
                pvt()
                pend.pop(0)
                popped += 1
                if fin:
                    fint()
                    cooldown[0] = 3
                    break

        for qq in range(QQ):
            o_ps = [
                psO.tile([HS + 1, QV], FP32, tag="o", name=f"o{h}")
                for h in range(2)
            ]
            un2 = un_pool.tile([128, QV], BF16, tag="un", name="un2")

            def pv(mc, p_sb, o_ps_=o_ps):
                for h in range(2):
                    nc.tensor.matmul(
                        o_ps_[h][:],
                        vn[:, mc, h, :],
                        p_sb[:, h * QV : (h + 1) * QV],
                        start=(mc == 0),
                        stop=(mc == MC - 1),
                    )

            def fin_q(qq_=qq, o_ps_=o_ps, un_t=un2):
                if qq_ < QQ - 1:
                    cpy, rbmm, recip, muls = tail_steps(qq_, o_ps_, un_t)
                    cpy()
                    rbmm()
                    # recip/muls next unit (frees o_ps before the cooldown
                    # ends); finals spaced so each f matmul's pp-slot wait
                    # (two-ago f's ob cast) is long done when it pops
                    fs = [
                        (lambda c=c: emit_final(qq_, un_t, c))
                        for c in range(4)
                    ]
                    btail.extend(
                        [[recip, muls], [], [fs[0]], [], [fs[1]], [],
                         [fs[2]], [], [fs[3]]]
                    )
                else:
                    tail_last(qq_, o_ps_, un_t)

            for mc in range(MC):
                u = qq * MC + mc
                s2 = s2d.pop((qq, mc))
                p_sb = pt_pool.tile([128, 1024], BF16, tag="p", name="p_sb")
                nc.scalar.activation(
                    p_sb[:], s2[:], mybir.ActivationFunctionType.Exp
                )
                if mc < MC - 1:
                    if (qq, mc + 1) not in s2d:
                        emit_S(qq, mc + 1)
                elif qq < QQ - 1:
                    emit_S(qq + 1, 0)
                if btail:
                    for fn in btail.pop(0):
                        fn()
                elif deferred:
                    deferred.pop(0)()
                pend.append(
                    (mc == MC - 1, lambda m=mc, p=p_sb, f=pv: f(m, p), fin_q)
                )
                if cooldown[0] > 0:
                    cooldown[0] -= 1
                elif len(pend) > lag_of(u):
                    cap = 1 if btail else 2
                    pop_pend(min(cap, len(pend) - lag_of(u)))

        # drain: remaining PVs of the last quarter, then its inline tail
        while pend:
            pop_pend(4)
        while btail:
            for fn in btail.pop(0):
                fn()
    if finalize:
        nc.finalize()
    return nc


_NC_CACHE = None


def _get_nc():
    global _NC_CACHE
    if _NC_CACHE is None:
        _NC_CACHE = build_nc()
    return _NC_CACHE


def _prep_xt(x, dt):
    # [N, D] fp32 -> [128, NS, DC, SLAB] slab-major:
    # xt[p, j, c, n'] = x[j*SLAB + n', c*128 + p]
    return np.ascontiguousarray(
        x.reshape(NS, SLAB, DC, 128).transpose(3, 0, 2, 1)
    ).astype(dt)


def _prep_xk(x, dt):
    # [N, D] fp32 -> [128, MC, DC, 128] mc-major:
    # xt[p, mc, c, n'] = x[mc*128 + n', c*128 + p]
    return np.ascontiguousarray(
        x.reshape(MC, 128, DC, 128).transpose(3, 0, 2, 1)
    ).astype(dt)


def _prep_w(w2, dt):
    # [D, 128] -> [128, DC, 128] with w[p, c, h] = w2[c*128+p, h]
    return np.ascontiguousarray(
        w2.reshape(DC, 128, 128).transpose(1, 0, 2)
    ).astype(dt)


def make_in_maps(inputs):
    query = np.asarray(inputs["query"], np.float32)
    key = np.asarray(inputs["key"], np.float32)
    value = np.asarray(inputs["value"], np.float32)
    Wq = np.asarray(inputs["Wq"], np.float32) / np.sqrt(np.float32(HS))
    Wk = np.asarray(inputs["Wk"], np.float32)
    Wv = np.asarray(inputs["Wv"], np.float32)
    Wp = np.asarray(inputs["Wp"], np.float32)

    in_maps = []
    for c in range(NCORES):
        b = c // 4
        h0 = 2 * (c % 4)
        w_all = np.concatenate(
            [
                _prep_w(
                    np.concatenate([W[h0], W[h0 + 1]], axis=1), np.float32
                ).reshape(128, DC * 128)
                for W in (Wk, Wq, Wv)
            ]
            + [np.concatenate([Wp[h0], Wp[h0 + 1]], axis=0)],
            axis=1,
        ).astype(nbf16)
        in_maps.append(
            {
                "xq": _prep_xt(query[b], nbf16),
                "xk": _prep_xk(key[b], nbf16),
                "xv": _prep_xt(value[b], nbf16),
                "wall": np.ascontiguousarray(w_all),
            }
        )
    return in_maps


def kernel(query, key, value, Wq, Wk, Wv, Wp):
    in_maps = make_in_maps(
        dict(query=query, key=key, value=value, Wq=Wq, Wk=Wk, Wv=Wv, Wp=Wp)
    )
    nc = _get_nc()
    res = run_bass_kernel_spmd(nc, in_maps, list(range(NCORES)))
    out = np.zeros((B, N, D), np.float32)
    for c in range(NCORES):
        out[c // 4] += np.asarray(res.results[c]["out"], np.float32)
    return out


if __name__ == "__main__":
    d = np.load("/root/problem/work/ref.npz")
    got = kernel(
        d["query"], d["key"], d["value"], d["Wq"], d["Wk"], d["Wv"], d["Wp"]
    )
    exp = d["expected"]
    rel = np.linalg.norm(got - exp) / np.linalg.norm(exp)
    print("Relative error:", rel)
